# revision 29
# baseline (speedup 1.0000x reference)
"""Trainium2 Bass kernel for causal multi-head attention with RoPE.

Problem: x[2,2048,2048] -> qkv proj -> RoPE(q,k) -> causal softmax attention
(16 heads, hd=128) -> out proj.  Sharding: tensor-parallel over heads
(2 heads/core x 8 cores); the output projection contraction is restored
with one AllToAll per batch (head-shards -> sequence-shards), overlapped
with the other batch's compute, so each core computes a disjoint
[2, 256, 2048] slice of the final output.

All matmuls run as float32r (full-rate fp32 PE mode, ~1.6e-4 rel err on a
2048-deep contraction).  Softmax skips the max-subtraction (scores are
O(1) by construction); the causal mask is accumulated into PSUM as a
-1e9 constant via a PE identity-matmul; softmax denominators are
partition-reduced and broadcast back with tiny ones-matmuls on the PE.
"""

import os
import sys

if "/opt/trn_rl_repo" not in sys.path:
    sys.path.insert(0, "/opt/trn_rl_repo")

import numpy as np

B, S, D = 2, 2048, 2048
H, HD = 16, 128
NCORES = 8
HPC = H // NCORES          # heads per core (2)
ROPE_BASE = 10000.0
SCALE = 1.0 / float(np.sqrt(HD))
SC = 512                   # QKV matmul free-dim chunk (s positions)
KSUB = D // 128            # 16 contraction subtiles
SCW = S // NCORES          # 256: per-core output cols per batch

_CACHE = {}


def _install_trace_shim():
    """Optionally register the axon NTFF profile hook (for test.py tracing)."""
    try:
        import types

        if "antenv.axon_hooks" in sys.modules:
            return True
        import antenv
        from trn_agent_boot.trn_boot import _ntff_profile_via_ctypes

        hook = _ntff_profile_via_ctypes("/opt/axon/libaxon_pjrt.so")
        mod = types.ModuleType("antenv.axon_hooks")
        _state = {"hook": hook}
        mod.get_axon_ntff_profile_hook = lambda: _state["hook"]
        mod.set_axon_ntff_profile_hook = lambda h: _state.__setitem__("hook", h)
        sys.modules["antenv.axon_hooks"] = mod
        antenv.axon_hooks = mod
        return True
    except Exception:
        return False


def _build():
    import concourse.bass as bass  # noqa: F401
    import concourse.mybir as mybir
    import concourse.tile as tile
    from concourse import bacc
    from concourse.masks import make_identity

    f32 = mybir.dt.float32
    f32r = mybir.dt.float32r
    bf16 = mybir.dt.bfloat16
    EXP = mybir.ActivationFunctionType.Exp
    CPY = mybir.ActivationFunctionType.Copy

    nc = bacc.Bacc("TRN2", target_bir_lowering=False, debug=False,
                   num_devices=NCORES)

    xT = nc.dram_tensor("xT", [128, KSUB, B * S], bf16, kind="ExternalInput")
    wqkv = nc.dram_tensor("wqkv", [128, KSUB, 3 * HPC * HD], bf16,
                          kind="ExternalInput")
    wout = nc.dram_tensor("wout", [128, KSUB, D], bf16, kind="ExternalInput")
    cosg = nc.dram_tensor("cosg", [128, S], bf16, kind="ExternalInput")
    sing = nc.dram_tensor("sing", [128, S], bf16, kind="ExternalInput")
    mneg = nc.dram_tensor("mneg", [128, 512], bf16, kind="ExternalInput")
    y = nc.dram_tensor("y", [B, SCW, D], f32, kind="ExternalOutput")

    NQC = S // SC          # qkv s-chunks per batch
    NKT = S // 128         # 16 key tiles
    VOFF = 2 * HPC * HD    # v block column offset in w_sb (512)
    MORD = (2, 0, 3, 1)    # qk head-slice order: k0, q0, k1, q1

    with tile.TileContext(nc) as tc:
        with tc.tile_pool(name="const", bufs=1) as cp, \
             tc.tile_pool(name="dram", bufs=1, space="DRAM") as dp, \
             tc.tile_pool(name="psA", bufs=6, space="PSUM") as psA, \
             tc.tile_pool(name="psOut", bufs=2, space="PSUM") as psO, \
             tc.tile_pool(name="w", bufs=1) as wp, \
             tc.tile_pool(name="xc", bufs=4) as xp, \
             tc.tile_pool(name="qkv", bufs=1) as qp, \
             tc.tile_pool(name="attn", bufs=1) as ap_, \
             tc.tile_pool(name="rotp", bufs=1) as rp, \
             tc.tile_pool(name="wo3p", bufs=1) as wo3p, \
             tc.tile_pool(name="stage", bufs=1) as stp, \
             tc.tile_pool(name="small", bufs=8) as ep:

            cos_sb = cp.tile([128, S], bf16, name="cos_sb")
            sin_sb = cp.tile([128, S], bf16, name="sin_sb")
            mneg_sb = cp.tile([128, 512], bf16, name="mneg_sb")
            ident = cp.tile([128, 128], f32, name="ident")
            identR = cp.tile([128, 128], bf16, name="identR")
            onesc = cp.tile([128, 1], f32, name="onesc")
            onescR = cp.tile([128, 1], f32r, name="onescR")
            onesr = cp.tile([1, 128], f32, name="onesr")
            onesrR = cp.tile([1, 128], f32r, name="onesrR")

            ibs = {(b, h, half): dp.tile([NCORES, 128, 128], bf16,
                                         name=f"ib{b}{h}{half}")
                   for b in range(B) for h in range(HPC) for half in (0, 1)}
            obs = {(b, h, half): dp.tile([NCORES, 128, 128], bf16,
                                         name=f"ob{b}{h}{half}")
                   for b in range(B) for h in range(HPC) for half in (0, 1)}

            def load_xc(b, sc, ksplit=4):
                """Chunked x load: split over k groups so matmuls can start
                as soon as the first k-slices land."""
                xc = xp.tile([128, KSUB, SC], bf16, tag="xc", name="xc")
                off = b * S + sc * SC
                kg = KSUB // ksplit
                for g in range(ksplit):
                    nc.sync.dma_start(
                        xc[:, g * kg:(g + 1) * kg, :],
                        xT.ap()[:, g * kg:(g + 1) * kg, off:off + SC])
                return xc

            # startup: first qk weight + first x chunk first (critical path),
            # remaining weights next, rope/mask constants last (needed much
            # later, issued on the otherwise-idle vector/scalar queues).
            wqk_t = [None] * (2 * HPC)
            wqk_t[MORD[0]] = wp.tile([128, KSUB, 128], bf16,
                                     tag=f"w{MORD[0]}", name=f"w{MORD[0]}")
            nc.sync.dma_start(
                wqk_t[MORD[0]][:],
                wqkv.ap()[:, :, MORD[0] * 128:(MORD[0] + 1) * 128])
            wv_t = wp.tile([128, KSUB, HPC * HD], bf16, tag="wv", name="wv")
            nc.gpsimd.dma_start(wv_t[:], wqkv.ap()[:, :, VOFF:VOFF + HPC * HD])
            xc0 = load_xc(0, 0)
            for m in MORD[1:]:
                wt = wp.tile([128, KSUB, 128], bf16, tag=f"w{m}", name=f"w{m}")
                eng = nc.sync if m == MORD[1] else nc.gpsimd
                eng.dma_start(wt[:], wqkv.ap()[:, :, m * 128:(m + 1) * 128])
                wqk_t[m] = wt

            make_identity(nc, ident[:])
            nc.vector.tensor_copy(identR[:], ident[:])
            nc.vector.memset(onesc[:], 1.0)
            nc.vector.tensor_copy(onescR[:], onesc[:])
            nc.vector.memset(onesr[:], 1.0)
            nc.vector.tensor_copy(onesrR[:], onesr[:])
            nc.scalar.dma_start(mneg_sb[:], mneg.ap())
            rope_consts_loaded = [False]

            def load_rope_consts():
                if rope_consts_loaded[0]:
                    return
                rope_consts_loaded[0] = True
                nc.scalar.dma_start(cos_sb[:], cosg.ap())
                nc.scalar.dma_start(sin_sb[:], sing.ap())

            def qkv_rope(b, pre_xc=None):
                qkT = qp.tile([128, 2 * HPC, S], bf16, tag="qkT")
                Vn = qp.tile([128, NKT, HPC * HD], bf16, tag="Vn")

                def emit_v(xc, sc):
                    for st2 in range(SC // 128):
                        ps = psA.tile([128, 512], f32, tag="bank")
                        for k in range(KSUB):
                            nc.tensor.matmul(
                                ps[:, :HPC * HD],
                                xc[:, k, st2 * 128:(st2 + 1) * 128],
                                wv_t[:, k],
                                start=(k == 0), stop=(k == KSUB - 1))
                        nc.vector.tensor_copy(
                            Vn[:, sc * (SC // 128) + st2], ps[:, :HPC * HD])

                def emit_qk(xc, sc, m):
                    ps = psA.tile([128, 512], f32, tag="bank")
                    for k in range(KSUB):
                        nc.tensor.matmul(
                            ps[:, :SC],
                            wqk_t[m][:, k],
                            xc[:, k],
                            start=(k == 0), stop=(k == KSUB - 1))
                    nc.vector.tensor_copy(
                        qkT[:, m, sc * SC:(sc + 1) * SC], ps[:, :SC])

                def rope(m):
                    # fused halves (sin grid stored pre-swapped):
                    # rt[0:64] = t[64:128]*(-sin); rt[64:128] = t[0:64]*sin;
                    # t *= cos; t += rt -- all bf16 so the DVE runs in its
                    # packed 2-byte high-rate mode
                    rt = rp.tile([128, S], bf16, tag="rot", name="rt")
                    nc.vector.tensor_mul(rt[0:64, :],
                                         qkT[64:128, m],
                                         sin_sb[64:128, :])
                    nc.vector.tensor_mul(rt[64:128, :],
                                         qkT[0:64, m],
                                         sin_sb[0:64, :])
                    nc.vector.tensor_mul(qkT[:, m], qkT[:, m], cos_sb[:])
                    nc.vector.tensor_add(qkT[:, m], qkT[:, m], rt[:])

                if pre_xc is not None:
                    # batch 0: x streams in chunk by chunk; V first per
                    # chunk except the last, where all four ropes run on
                    # DVE under the remaining qk+V matmuls
                    for sc in range(NQC - 1):
                        if sc == 0:
                            xc = pre_xc
                            emit_qk(xc, sc, MORD[0])
                            emit_qk(xc, sc, MORD[1])
                            emit_v(xc, sc)
                            emit_qk(xc, sc, MORD[2])
                            emit_qk(xc, sc, MORD[3])
                            continue
                        xc = load_xc(b, sc)
                        load_rope_consts()
                        emit_v(xc, sc)
                        for m in MORD:
                            emit_qk(xc, sc, m)
                    sc = NQC - 1
                    xc = load_xc(b, sc)
                    emit_qk(xc, sc, MORD[0])
                    rope(MORD[0])
                    emit_qk(xc, sc, MORD[1])
                    rope(MORD[1])
                    emit_qk(xc, sc, MORD[2])
                    emit_qk(xc, sc, MORD[3])
                    emit_v(xc, sc)

                    def rope_op(m, step, rt):
                        if step == 0:
                            nc.vector.tensor_mul(rt[0:64, :], qkT[64:128, m],
                                                 sin_sb[64:128, :])
                        elif step == 1:
                            nc.vector.tensor_mul(rt[64:128, :], qkT[0:64, m],
                                                 sin_sb[0:64, :])
                        elif step == 2:
                            nc.vector.tensor_mul(qkT[:, m], qkT[:, m],
                                                 cos_sb[:])
                        else:
                            nc.vector.tensor_add(qkT[:, m], qkT[:, m], rt[:])

                    fs = []
                    for m in (MORD[2], MORD[3]):
                        rt = rp.tile([128, S], bf16, tag="rot2", name="rt2")
                        fs.append(m)
                        fs[-1:] = [(lambda m=m, s=s, rt=rt:
                                    rope_op(m, s, rt)) for s in range(4)]
                    return qkT, Vn, fs
                else:
                    # batch 1: x was prefetched during batch 0's attention;
                    # head-outer order lets each rope hide under the next
                    # head's matmuls, and the V phase runs rope-free
                    xcs = [load_xc(b, sc) for sc in range(NQC)]
                    for m in MORD:
                        for sc in range(NQC):
                            emit_qk(xcs[sc], sc, m)
                        rope(m)
                    for sc in range(NQC):
                        emit_v(xcs[sc], sc)
                return qkT, Vn, []

            pending_ship = []

            def attention(b, h, qkT, Vn, fillers=()):
                """Query-chunk-outer causal attention.

                For each 512-query chunk qc, loop key tiles kt=0..4*qc+3:
                score matmul -> exp (Act) -> AV accumulate into one PSUM
                bank -> denominator adds on DVE (two parity chains).  Scores
                run LAG rounds ahead through 6 rotating psA banks so the Act
                engine never starves.  Each chunk's finalize (which must
                wait on the DVE add chains) is deferred into round 2 of the
                NEXT chunk so it never stalls the in-order PE queue.
                """
                fillers = list(fillers)
                pending_fin = []
                st = rp.tile([128, S], bf16, tag="rot", name="st")
                accE = ap_.tile([128, S], f32r, tag="accE", name="accE")
                accO = ap_.tile([128, S], f32r, tag="accO", name="accO")
                LAG = 2

                def make_finalize(j, oT, ship):
                    def emit():
                        # normalize: partition-reduce both parity chains
                        # into row 0 of the finished outT bank, reciprocal,
                        # K=1 broadcast matmul back, scale, ship.
                        sl = slice(j * 512, (j + 1) * 512)
                        nc.scalar.activation(st[:, sl], oT[:], CPY)
                        nc.tensor.matmul(oT[0:1, :], onescR[:], accE[:, sl],
                                         start=True, stop=False)
                        nc.tensor.matmul(oT[0:1, :], onescR[:], accO[:, sl],
                                         start=False, stop=True)
                        srow = stp.tile([1, 512], f32, tag="srow",
                                        name="srow")
                        nc.vector.reciprocal_approx_fast(srow[:], oT[0:1, :])
                        srowr = stp.tile([1, 512], f32r, tag="srowr",
                                         name="srowr")
                        nc.vector.tensor_copy(srowr[:], srow[:])
                        nc.tensor.matmul(oT[:], onesrR[:], srowr[:],
                                         start=True, stop=True)
                        nc.vector.tensor_mul(st[:, sl], st[:, sl], oT[:])
                        half, base = (0, 0) if j < 2 else (1, 8)
                        for blk in range(4 * j, 4 * j + 4):
                            nc.sync.dma_start(
                                ibs[(b, h, half)][blk - 8 * half],
                                st[:, blk * 128:(blk + 1) * 128])
                        if ship is not None:
                            # defer the collective a couple of rounds so it
                            # never sits in front of the Pool-queue inits
                            pending_ship.append(ship)
                    return emit

                for qc in range(4):
                    R = 4 * qc + 4
                    qs = qc * 512
                    oT = psO.tile([128, 512], f32, tag="outT")
                    sps = {}

                    def emit_score(kt):
                        sp = psA.tile([128, 512], f32, tag="bank")
                        sps[kt] = sp
                        if kt // 4 == qc:      # diagonal tile: causal mask
                            o = 128 * (kt - 4 * qc)
                            nc.tensor.matmul(sp[:, o:512], identR[:],
                                             mneg_sb[:, 0:512 - o],
                                             start=True, stop=False)
                            nc.tensor.matmul(
                                sp[:, o:512],
                                qkT[:, HPC + h, kt * 128:(kt + 1) * 128],
                                qkT[:, h, qs + o:qs + 512],
                                start=False, stop=True)
                        else:
                            nc.tensor.matmul(
                                sp[:, :],
                                qkT[:, HPC + h, kt * 128:(kt + 1) * 128],
                                qkT[:, h, qs:qs + 512],
                                start=True, stop=True)

                    for r in range(min(LAG, R)):
                        emit_score(r)
                    for r in range(R):
                        if r + LAG < R:
                            emit_score(r + LAG)
                        o = 128 * (r - 4 * qc) if r // 4 == qc else 0
                        sp = sps.pop(r)
                        et = ep.tile([128, 512], bf16, tag="expT")
                        nc.scalar.activation(et[:, o:512], sp[:, o:512],
                                             EXP, scale=SCALE)
                        nc.tensor.matmul(
                            oT[:, o:512],
                            Vn[:, r, h * 128:(h + 1) * 128],
                            et[:, o:512],
                            start=(r == 0), stop=(r == R - 1))
                        acc = accE if r % 2 == 0 else accO
                        if r < 2:
                            # parity-chain init; round 1 of chunk 0 is a
                            # masked diagonal round, so only [o:512] of et
                            # is valid -- zero the rest
                            if o:
                                nc.gpsimd.memset(
                                    acc[:, qs:qs + o].bitcast(f32), 0.0)
                            nc.scalar.activation(acc[:, qs + o:qs + 512],
                                                 et[:, o:512], CPY)
                        else:
                            nc.vector.tensor_add(acc[:, qs + o:qs + 512],
                                                 acc[:, qs + o:qs + 512],
                                                 et[:, o:512])
                        if r == 2 and pending_fin:
                            pending_fin.pop(0)()
                        if r == 3 and pending_ship:
                            pending_ship.pop(0)()
                        if fillers and (
                                (r == 3 and qc >= 1) or
                                (qc >= 2 and r == 4 * qc - 2)):
                            fillers.pop(0)()
                    ship = (a2a_for.get((b, h, 0)) if qc == 1 else
                            a2a_for.get((b, h, 1)) if qc == 3 else None)
                    pending_fin.append(make_finalize(qc, oT, ship))
                while pending_fin:
                    pending_fin.pop(0)()
                while fillers:
                    fillers.pop(0)()

            def drain_ship():
                while pending_ship:
                    pending_ship.pop(0)()

            def load_lhs(b, pool, tag, halves=(0, 1)):
                # k-subtile order hh*8+i <-> global head 2i+hh (wout is
                # permuted host-side to match); output query blocks c and
                # c+8 land in column halves 0:128 / 128:256
                lhs = pool.tile([128, KSUB, SCW], bf16, tag=tag,
                                name=f"lhs{b}")
                for hh in range(HPC):
                    for half in halves:
                        nc.scalar.dma_start(
                            lhs[:, hh * NCORES:(hh + 1) * NCORES,
                                half * 128:(half + 1) * 128],
                            obs[(b, hh, half)][:].rearrange("i p s -> p i s"))
                return lhs

            def outproj_group(b, n, m, lhs, wo, wocol, on_act=False):
                ps = psA.tile([128, 512], f32, tag="bank")
                for k in range(KSUB):
                    nc.tensor.matmul(
                        ps[:],
                        lhs[:, k, m * 128:(m + 1) * 128],
                        wo[:, k, wocol * 512:(wocol + 1) * 512],
                        start=(k == 0), stop=(k == KSUB - 1))
                ys = ep.tile([128, 512], f32, tag="expT", name="ys")
                if on_act:
                    nc.scalar.activation(ys[:], ps[:], CPY)
                else:
                    nc.vector.tensor_copy(ys[:], ps[:])
                nc.sync.dma_start(
                    y.ap()[b, m * 128:(m + 1) * 128,
                           n * 512:(n + 1) * 512],
                    ys[:])

            def a2a(b, h, half):
                nc.gpsimd.collective_compute(
                    "AllToAll", mybir.AluOpType.bypass,
                    replica_groups=[list(range(NCORES))],
                    ins=[ibs[(b, h, half)].opt()],
                    outs=[obs[(b, h, half)].opt()])

            # batch 0 compute; its A2A runs while batch 1 computes;
            # outproj(0) slots into PE after batch 1's attention.
            a2a_for = {(b, h, half): (lambda b=b, h=h, half=half:
                                      a2a(b, h, half))
                       for b in range(B) for h in range(HPC)
                       for half in (0, 1)}
            qkT, Vn, rfs = qkv_rope(0, pre_xc=xc0)
            attention(0, 0, qkT, Vn, fillers=rfs)
            attention(0, 1, qkT, Vn)
            qkT, Vn, _ = qkv_rope(1)
            attention(1, 0, qkT, Vn)

            # prefetch first two wout column-chunks into the (now dead) x
            # chunk slots while attention(1,1) runs
            wos = {}
            for n in (0, 1):
                wo = xp.tile([128, KSUB, 512], bf16, tag="xc", name="wo")
                nc.scalar.dma_start(wo[:],
                                    wout.ap()[:, :, n * 512:(n + 1) * 512])
                wos[n] = wo
            drain_ship()
            lhs0 = load_lhs(0, wp, "wv")

            attention(1, 1, qkT, Vn)

            # wout chunks 2/3 ride in the dead qkT slot and a small
            # dedicated slot
            wos[2] = qp.tile([128, KSUB, 512], bf16, tag="qkT", name="wo2")
            nc.scalar.dma_start(wos[2][:], wout.ap()[:, :, 1024:1536])
            wos[3] = wo3p.tile([128, KSUB, 512], bf16, tag="wo3", name="wo3")
            nc.scalar.dma_start(wos[3][:], wout.ap()[:, :, 1536:2048])
            drain_ship()
            lhs1 = load_lhs(1, qp, "Vn")

            # all batch-0 groups run while the final A2As land; batch-1
            # m=0 groups need only the half-A payload (shipped mid-
            # attention), m=1 groups run last.
            for n in range(4):
                for m in range(SCW // 128):
                    outproj_group(0, n, m, lhs0, wos[n], 0)
            for m in range(SCW // 128):
                for n in range(4):
                    outproj_group(1, n, m, lhs1, wos[n], 0)

    nc.finalize()
    return nc


def _host_inputs(x, w_qkv, w_out):
    import ml_dtypes
    bf16 = ml_dtypes.bfloat16
    xTr = np.ascontiguousarray(
        x.reshape(B * S, D).T.reshape(KSUB, 128, B * S).transpose(1, 0, 2)
    ).astype(bf16)
    horder = [2 * i + hh for hh in range(HPC) for i in range(NCORES)]
    woutr = np.ascontiguousarray(
        w_out.reshape(H, HD, D)[horder].transpose(1, 0, 2)).astype(bf16)

    half = HD // 2
    inv = (1.0 / (ROPE_BASE ** (np.arange(half, dtype=np.float32) / half))
           ).astype(np.float32)
    ang = (np.arange(S, dtype=np.float32)[:, None] * inv[None, :])  # [S, 64]
    c = np.cos(ang).astype(np.float32).T      # [64, S]
    s = np.sin(ang).astype(np.float32).T
    cosg = np.ascontiguousarray(np.concatenate([c, c], axis=0)).astype(bf16)
    # pre-swapped: rows 0:64 = +sin (consumed against t[0:64] -> rt[64:128]),
    # rows 64:128 = -sin (consumed against t[64:128] -> rt[0:64])
    sing = np.ascontiguousarray(np.concatenate([s, -s], axis=0)).astype(bf16)

    # mneg[p, j] = 0 where j >= p else -1e9 (upper-tri of the diagonal
    # 128-block, padded to 512 query columns).
    u = np.arange(512)[None, :]
    p = np.arange(128)[:, None]
    mneg = np.where(u >= p, 0.0, -1e9).astype(bf16)

    maps = []
    for i in range(NCORES):
        h0, h1 = 2 * i, 2 * i + 1
        blocks = []
        for base in (0, D, 2 * D):
            blocks.append(w_qkv[:, base + 128 * h0:base + 128 * (h0 + 1)])
            blocks.append(w_qkv[:, base + 128 * h1:base + 128 * (h1 + 1)])
        shard = np.concatenate(blocks, axis=1)  # [D, 768]
        shard = np.ascontiguousarray(
            shard.reshape(KSUB, 128, 3 * HPC * HD).transpose(1, 0, 2)
        ).astype(bf16)
        maps.append({"xT": xTr, "wqkv": shard, "wout": woutr,
                     "cosg": cosg, "sing": sing, "mneg": mneg})
    return maps


def kernel(x, w_qkv, w_out):
    from concourse.bass_utils import run_bass_kernel_spmd

    x = np.asarray(x, dtype=np.float32)
    w_qkv = np.asarray(w_qkv, dtype=np.float32)
    w_out = np.asarray(w_out, dtype=np.float32)

    if "nc" not in _CACHE:
        _CACHE["nc"] = _build()
    nc = _CACHE["nc"]

    trace = bool(int(os.environ.get("KERNEL_TRACE", "0")))
    if trace:
        trace = _install_trace_shim()

    in_maps = _host_inputs(x, w_qkv, w_out)
    res = run_bass_kernel_spmd(nc, in_maps, core_ids=list(range(NCORES)),
                               trace=trace)
    _CACHE["last_result"] = res
    # y per core i: [B, 256, D]; rows 0:128 = query block i, rows
    # 128:256 = query block i+8 (per batch)
    full = np.empty((B * S, D), dtype=np.float32)
    for i in range(NCORES):
        yi = res.results[i]["y"]
        for b in range(B):
            full[b * S + i * 128: b * S + (i + 1) * 128] = yi[b, :128]
            full[b * S + 1024 + i * 128:
                 b * S + 1024 + (i + 1) * 128] = yi[b, 128:]
    return full.reshape(B, S, D)


# revision 30
# speedup vs baseline: 1.0135x; 1.0135x over previous
"""Trainium2 Bass kernel for causal multi-head attention with RoPE.

Problem: x[2,2048,2048] -> qkv proj -> RoPE(q,k) -> causal softmax attention
(16 heads, hd=128) -> out proj.  Sharding: tensor-parallel over heads
(2 heads/core x 8 cores); the output projection contraction is restored
with one AllToAll per batch (head-shards -> sequence-shards), overlapped
with the other batch's compute, so each core computes a disjoint
[2, 256, 2048] slice of the final output.

All matmuls run as float32r (full-rate fp32 PE mode, ~1.6e-4 rel err on a
2048-deep contraction).  Softmax skips the max-subtraction (scores are
O(1) by construction); the causal mask is accumulated into PSUM as a
-1e9 constant via a PE identity-matmul; softmax denominators are
partition-reduced and broadcast back with tiny ones-matmuls on the PE.
"""

import os
import sys

if "/opt/trn_rl_repo" not in sys.path:
    sys.path.insert(0, "/opt/trn_rl_repo")

import numpy as np

B, S, D = 2, 2048, 2048
H, HD = 16, 128
NCORES = 8
HPC = H // NCORES          # heads per core (2)
ROPE_BASE = 10000.0
SCALE = 1.0 / float(np.sqrt(HD))
SC = 512                   # QKV matmul free-dim chunk (s positions)
KSUB = D // 128            # 16 contraction subtiles
SCW = S // NCORES          # 256: per-core output cols per batch

_CACHE = {}


def _install_trace_shim():
    """Optionally register the axon NTFF profile hook (for test.py tracing)."""
    try:
        import types

        if "antenv.axon_hooks" in sys.modules:
            return True
        import antenv
        from trn_agent_boot.trn_boot import _ntff_profile_via_ctypes

        hook = _ntff_profile_via_ctypes("/opt/axon/libaxon_pjrt.so")
        mod = types.ModuleType("antenv.axon_hooks")
        _state = {"hook": hook}
        mod.get_axon_ntff_profile_hook = lambda: _state["hook"]
        mod.set_axon_ntff_profile_hook = lambda h: _state.__setitem__("hook", h)
        sys.modules["antenv.axon_hooks"] = mod
        antenv.axon_hooks = mod
        return True
    except Exception:
        return False


def _build():
    import concourse.bass as bass  # noqa: F401
    import concourse.mybir as mybir
    import concourse.tile as tile
    from concourse import bacc
    from concourse.masks import make_identity

    f32 = mybir.dt.float32
    f32r = mybir.dt.float32r
    bf16 = mybir.dt.bfloat16
    EXP = mybir.ActivationFunctionType.Exp
    CPY = mybir.ActivationFunctionType.Copy

    nc = bacc.Bacc("TRN2", target_bir_lowering=False, debug=False,
                   num_devices=NCORES)

    xT = nc.dram_tensor("xT", [128, KSUB, B * S], bf16, kind="ExternalInput")
    wqkv = nc.dram_tensor("wqkv", [128, KSUB, 3 * HPC * HD], bf16,
                          kind="ExternalInput")
    wout = nc.dram_tensor("wout", [128, KSUB, D], bf16, kind="ExternalInput")
    cosg = nc.dram_tensor("cosg", [128, S], bf16, kind="ExternalInput")
    sing = nc.dram_tensor("sing", [128, S], bf16, kind="ExternalInput")
    mneg = nc.dram_tensor("mneg", [128, 512], bf16, kind="ExternalInput")
    y = nc.dram_tensor("y", [B, SCW, D], f32, kind="ExternalOutput")

    NQC = S // SC          # qkv s-chunks per batch
    NKT = S // 128         # 16 key tiles
    VOFF = 2 * HPC * HD    # v block column offset in w_sb (512)
    MORD = (2, 0, 3, 1)    # qk head-slice order: k0, q0, k1, q1

    with tile.TileContext(nc) as tc:
        with tc.tile_pool(name="const", bufs=1) as cp, \
             tc.tile_pool(name="dram", bufs=1, space="DRAM") as dp, \
             tc.tile_pool(name="psA", bufs=6, space="PSUM") as psA, \
             tc.tile_pool(name="psOut", bufs=2, space="PSUM") as psO, \
             tc.tile_pool(name="w", bufs=1) as wp, \
             tc.tile_pool(name="xc", bufs=4) as xp, \
             tc.tile_pool(name="qkv", bufs=1) as qp, \
             tc.tile_pool(name="attn", bufs=1) as ap_, \
             tc.tile_pool(name="rotp", bufs=1) as rp, \
             tc.tile_pool(name="wo3p", bufs=1) as wo3p, \
             tc.tile_pool(name="stage", bufs=1) as stp, \
             tc.tile_pool(name="small", bufs=8) as ep:

            cos_sb = cp.tile([128, S], bf16, name="cos_sb")
            sin_sb = cp.tile([128, S], bf16, name="sin_sb")
            mneg_sb = cp.tile([128, 512], bf16, name="mneg_sb")
            ident = cp.tile([128, 128], f32, name="ident")
            identR = cp.tile([128, 128], bf16, name="identR")
            onesc = cp.tile([128, 1], f32, name="onesc")
            onescR = cp.tile([128, 1], f32r, name="onescR")
            onesr = cp.tile([1, 128], f32, name="onesr")
            onesrR = cp.tile([1, 128], f32r, name="onesrR")

            ibs = {(b, h, half): dp.tile([NCORES, 128, 128], bf16,
                                         name=f"ib{b}{h}{half}")
                   for b in range(B) for h in range(HPC) for half in (0, 1)}
            obs = {(b, h, half): dp.tile([NCORES, 128, 128], bf16,
                                         name=f"ob{b}{h}{half}")
                   for b in range(B) for h in range(HPC) for half in (0, 1)}

            def load_xc(b, sc, ksplit=4):
                """Chunked x load: split over k groups so matmuls can start
                as soon as the first k-slices land."""
                xc = xp.tile([128, KSUB, SC], bf16, tag="xc", name="xc")
                off = b * S + sc * SC
                kg = KSUB // ksplit
                for g in range(ksplit):
                    nc.sync.dma_start(
                        xc[:, g * kg:(g + 1) * kg, :],
                        xT.ap()[:, g * kg:(g + 1) * kg, off:off + SC])
                return xc

            # startup: first qk weight + first x chunk first (critical path),
            # remaining weights next, rope/mask constants last (needed much
            # later, issued on the otherwise-idle vector/scalar queues).
            wqk_t = [None] * (2 * HPC)
            wqk_t[MORD[0]] = wp.tile([128, KSUB, 128], bf16,
                                     tag=f"w{MORD[0]}", name=f"w{MORD[0]}")
            nc.sync.dma_start(
                wqk_t[MORD[0]][:],
                wqkv.ap()[:, :, MORD[0] * 128:(MORD[0] + 1) * 128])
            wv_t = wp.tile([128, KSUB, HPC * HD], bf16, tag="wv", name="wv")
            nc.gpsimd.dma_start(wv_t[:], wqkv.ap()[:, :, VOFF:VOFF + HPC * HD])
            xc0 = load_xc(0, 0, ksplit=8)
            for m in MORD[1:]:
                wt = wp.tile([128, KSUB, 128], bf16, tag=f"w{m}", name=f"w{m}")
                eng = nc.sync if m == MORD[1] else nc.gpsimd
                eng.dma_start(wt[:], wqkv.ap()[:, :, m * 128:(m + 1) * 128])
                wqk_t[m] = wt

            make_identity(nc, ident[:])
            nc.vector.tensor_copy(identR[:], ident[:])
            nc.vector.memset(onesc[:], 1.0)
            nc.vector.tensor_copy(onescR[:], onesc[:])
            nc.vector.memset(onesr[:], 1.0)
            nc.vector.tensor_copy(onesrR[:], onesr[:])
            nc.scalar.dma_start(mneg_sb[:], mneg.ap())
            rope_consts_loaded = [False]

            def load_rope_consts():
                if rope_consts_loaded[0]:
                    return
                rope_consts_loaded[0] = True
                nc.scalar.dma_start(cos_sb[:], cosg.ap())
                nc.scalar.dma_start(sin_sb[:], sing.ap())

            def qkv_rope(b, pre_xc=None):
                qkT = qp.tile([128, 2 * HPC, S], bf16, tag="qkT")
                Vn = qp.tile([128, NKT, HPC * HD], bf16, tag="Vn")

                def emit_v(xc, sc):
                    for st2 in range(SC // 128):
                        ps = psA.tile([128, 512], f32, tag="bank")
                        for k in range(KSUB):
                            nc.tensor.matmul(
                                ps[:, :HPC * HD],
                                xc[:, k, st2 * 128:(st2 + 1) * 128],
                                wv_t[:, k],
                                start=(k == 0), stop=(k == KSUB - 1))
                        nc.vector.tensor_copy(
                            Vn[:, sc * (SC // 128) + st2], ps[:, :HPC * HD])

                def emit_qk(xc, sc, m):
                    ps = psA.tile([128, 512], f32, tag="bank")
                    for k in range(KSUB):
                        nc.tensor.matmul(
                            ps[:, :SC],
                            wqk_t[m][:, k],
                            xc[:, k],
                            start=(k == 0), stop=(k == KSUB - 1))
                    nc.vector.tensor_copy(
                        qkT[:, m, sc * SC:(sc + 1) * SC], ps[:, :SC])

                def rope(m):
                    # fused halves (sin grid stored pre-swapped):
                    # rt[0:64] = t[64:128]*(-sin); rt[64:128] = t[0:64]*sin;
                    # t *= cos; t += rt -- all bf16 so the DVE runs in its
                    # packed 2-byte high-rate mode
                    rt = rp.tile([128, S], bf16, tag="rot", name="rt")
                    nc.vector.tensor_mul(rt[0:64, :],
                                         qkT[64:128, m],
                                         sin_sb[64:128, :])
                    nc.vector.tensor_mul(rt[64:128, :],
                                         qkT[0:64, m],
                                         sin_sb[0:64, :])
                    nc.vector.tensor_mul(qkT[:, m], qkT[:, m], cos_sb[:])
                    nc.vector.tensor_add(qkT[:, m], qkT[:, m], rt[:])

                if pre_xc is not None:
                    # batch 0: x streams in chunk by chunk; V first per
                    # chunk except the last, where all four ropes run on
                    # DVE under the remaining qk+V matmuls
                    for sc in range(NQC - 1):
                        if sc == 0:
                            xc = pre_xc
                            emit_qk(xc, sc, MORD[0])
                            emit_qk(xc, sc, MORD[1])
                            emit_v(xc, sc)
                            emit_qk(xc, sc, MORD[2])
                            emit_qk(xc, sc, MORD[3])
                            continue
                        xc = load_xc(b, sc)
                        load_rope_consts()
                        emit_v(xc, sc)
                        for m in MORD:
                            emit_qk(xc, sc, m)
                    sc = NQC - 1
                    xc = load_xc(b, sc)
                    emit_qk(xc, sc, MORD[0])
                    rope(MORD[0])
                    emit_qk(xc, sc, MORD[1])
                    rope(MORD[1])
                    emit_qk(xc, sc, MORD[2])
                    emit_qk(xc, sc, MORD[3])
                    emit_v(xc, sc)

                    def rope_op(m, step, rt):
                        if step == 0:
                            nc.vector.tensor_mul(rt[0:64, :], qkT[64:128, m],
                                                 sin_sb[64:128, :])
                        elif step == 1:
                            nc.vector.tensor_mul(rt[64:128, :], qkT[0:64, m],
                                                 sin_sb[0:64, :])
                        elif step == 2:
                            nc.vector.tensor_mul(qkT[:, m], qkT[:, m],
                                                 cos_sb[:])
                        else:
                            nc.vector.tensor_add(qkT[:, m], qkT[:, m], rt[:])

                    fs = []
                    for m in (MORD[2], MORD[3]):
                        rt = rp.tile([128, S], bf16, tag="rot2", name="rt2")
                        fs.append(m)
                        fs[-1:] = [(lambda m=m, s=s, rt=rt:
                                    rope_op(m, s, rt)) for s in range(4)]
                    return qkT, Vn, fs
                else:
                    # batch 1: x was prefetched during batch 0's attention;
                    # head-outer order lets each rope hide under the next
                    # head's matmuls, and the V phase runs rope-free
                    xcs = [load_xc(b, sc) for sc in range(NQC)]
                    for m in MORD:
                        for sc in range(NQC):
                            emit_qk(xcs[sc], sc, m)
                        rope(m)
                    for sc in range(NQC):
                        emit_v(xcs[sc], sc)
                return qkT, Vn, []

            pending_ship = []

            def attention(b, h, qkT, Vn, fillers=()):
                """Query-chunk-outer causal attention.

                For each 512-query chunk qc, loop key tiles kt=0..4*qc+3:
                score matmul -> exp (Act) -> AV accumulate into one PSUM
                bank -> denominator adds on DVE (two parity chains).  Scores
                run LAG rounds ahead through 6 rotating psA banks so the Act
                engine never starves.  Each chunk's finalize (which must
                wait on the DVE add chains) is deferred into round 2 of the
                NEXT chunk so it never stalls the in-order PE queue.
                """
                fillers = list(fillers)
                pending_fin = []
                st = rp.tile([128, S], bf16, tag="rot", name="st")
                accE = ap_.tile([128, S], f32r, tag="accE", name="accE")
                accO = ap_.tile([128, S], f32r, tag="accO", name="accO")
                LAG = 3

                def make_finalize(j, oT, ship):
                    def emit():
                        # normalize: partition-reduce both parity chains
                        # into row 0 of the finished outT bank, reciprocal,
                        # K=1 broadcast matmul back, scale, ship.
                        sl = slice(j * 512, (j + 1) * 512)
                        nc.scalar.activation(st[:, sl], oT[:], CPY)
                        nc.tensor.matmul(oT[0:1, :], onescR[:], accE[:, sl],
                                         start=True, stop=False)
                        nc.tensor.matmul(oT[0:1, :], onescR[:], accO[:, sl],
                                         start=False, stop=True)
                        srow = stp.tile([1, 512], f32, tag="srow",
                                        name="srow")
                        nc.vector.reciprocal_approx_fast(srow[:], oT[0:1, :])
                        srowr = stp.tile([1, 512], f32r, tag="srowr",
                                         name="srowr")
                        nc.vector.tensor_copy(srowr[:], srow[:])
                        nc.tensor.matmul(oT[:], onesrR[:], srowr[:],
                                         start=True, stop=True)
                        nc.vector.tensor_mul(st[:, sl], st[:, sl], oT[:])
                        half, base = (0, 0) if j < 2 else (1, 8)
                        for blk in range(4 * j, 4 * j + 4):
                            nc.sync.dma_start(
                                ibs[(b, h, half)][blk - 8 * half],
                                st[:, blk * 128:(blk + 1) * 128])
                        if ship is not None:
                            # defer the collective a couple of rounds so it
                            # never sits in front of the Pool-queue inits
                            pending_ship.append(ship)
                    return emit

                for qc in range(4):
                    R = 4 * qc + 4
                    qs = qc * 512
                    oT = psO.tile([128, 512], f32, tag="outT")
                    sps = {}

                    def emit_score(kt):
                        sp = psA.tile([128, 512], f32, tag="bank")
                        sps[kt] = sp
                        if kt // 4 == qc:      # diagonal tile: causal mask
                            o = 128 * (kt - 4 * qc)
                            nc.tensor.matmul(sp[:, o:512], identR[:],
                                             mneg_sb[:, 0:512 - o],
                                             start=True, stop=False)
                            nc.tensor.matmul(
                                sp[:, o:512],
                                qkT[:, HPC + h, kt * 128:(kt + 1) * 128],
                                qkT[:, h, qs + o:qs + 512],
                                start=False, stop=True)
                        else:
                            nc.tensor.matmul(
                                sp[:, :],
                                qkT[:, HPC + h, kt * 128:(kt + 1) * 128],
                                qkT[:, h, qs:qs + 512],
                                start=True, stop=True)

                    for r in range(min(LAG, R)):
                        emit_score(r)
                    for r in range(R):
                        if r + LAG < R:
                            emit_score(r + LAG)
                        o = 128 * (r - 4 * qc) if r // 4 == qc else 0
                        sp = sps.pop(r)
                        et = ep.tile([128, 512], bf16, tag="expT")
                        nc.scalar.activation(et[:, o:512], sp[:, o:512],
                                             EXP, scale=SCALE)
                        nc.tensor.matmul(
                            oT[:, o:512],
                            Vn[:, r, h * 128:(h + 1) * 128],
                            et[:, o:512],
                            start=(r == 0), stop=(r == R - 1))
                        acc = accE if r % 2 == 0 else accO
                        if r < 2:
                            # parity-chain init; round 1 of chunk 0 is a
                            # masked diagonal round, so only [o:512] of et
                            # is valid -- zero the rest
                            if o:
                                nc.gpsimd.memset(
                                    acc[:, qs:qs + o].bitcast(f32), 0.0)
                            nc.scalar.activation(acc[:, qs + o:qs + 512],
                                                 et[:, o:512], CPY)
                        else:
                            nc.vector.tensor_add(acc[:, qs + o:qs + 512],
                                                 acc[:, qs + o:qs + 512],
                                                 et[:, o:512])
                        if r == 2 and pending_fin:
                            pending_fin.pop(0)()
                        if r == 3 and pending_ship:
                            pending_ship.pop(0)()
                        if fillers and (
                                (r == 3 and qc >= 1) or
                                (qc >= 2 and r == 4 * qc - 2)):
                            fillers.pop(0)()
                    ship = (a2a_for.get((b, h, 0)) if qc == 1 else
                            a2a_for.get((b, h, 1)) if qc == 3 else None)
                    pending_fin.append(make_finalize(qc, oT, ship))
                while pending_fin:
                    pending_fin.pop(0)()
                while fillers:
                    fillers.pop(0)()

            def drain_ship():
                while pending_ship:
                    pending_ship.pop(0)()

            def load_lhs(b, pool, tag, halves=(0, 1)):
                # k-subtile order hh*8+i <-> global head 2i+hh (wout is
                # permuted host-side to match); output query blocks c and
                # c+8 land in column halves 0:128 / 128:256
                lhs = pool.tile([128, KSUB, SCW], bf16, tag=tag,
                                name=f"lhs{b}")
                for hh in range(HPC):
                    for half in halves:
                        nc.scalar.dma_start(
                            lhs[:, hh * NCORES:(hh + 1) * NCORES,
                                half * 128:(half + 1) * 128],
                            obs[(b, hh, half)][:].rearrange("i p s -> p i s"))
                return lhs

            def outproj_group(b, n, m, lhs, wo, wocol, on_act=False):
                ps = psA.tile([128, 512], f32, tag="bank")
                for k in range(KSUB):
                    nc.tensor.matmul(
                        ps[:],
                        lhs[:, k, m * 128:(m + 1) * 128],
                        wo[:, k, wocol * 512:(wocol + 1) * 512],
                        start=(k == 0), stop=(k == KSUB - 1))
                ys = ep.tile([128, 512], f32, tag="expT", name="ys")
                if on_act:
                    nc.scalar.activation(ys[:], ps[:], CPY)
                else:
                    nc.vector.tensor_copy(ys[:], ps[:])
                nc.sync.dma_start(
                    y.ap()[b, m * 128:(m + 1) * 128,
                           n * 512:(n + 1) * 512],
                    ys[:])

            def a2a(b, h, half):
                nc.gpsimd.collective_compute(
                    "AllToAll", mybir.AluOpType.bypass,
                    replica_groups=[list(range(NCORES))],
                    ins=[ibs[(b, h, half)].opt()],
                    outs=[obs[(b, h, half)].opt()])

            # batch 0 compute; its A2A runs while batch 1 computes;
            # outproj(0) slots into PE after batch 1's attention.
            a2a_for = {(b, h, half): (lambda b=b, h=h, half=half:
                                      a2a(b, h, half))
                       for b in range(B) for h in range(HPC)
                       for half in (0, 1)}
            qkT, Vn, rfs = qkv_rope(0, pre_xc=xc0)
            attention(0, 0, qkT, Vn, fillers=rfs)
            attention(0, 1, qkT, Vn)
            qkT, Vn, _ = qkv_rope(1)
            attention(1, 0, qkT, Vn)

            # prefetch first two wout column-chunks into the (now dead) x
            # chunk slots while attention(1,1) runs
            wos = {}
            for n in (0, 1):
                wo = xp.tile([128, KSUB, 512], bf16, tag="xc", name="wo")
                nc.scalar.dma_start(wo[:],
                                    wout.ap()[:, :, n * 512:(n + 1) * 512])
                wos[n] = wo
            drain_ship()
            lhs0 = load_lhs(0, wp, "wv")

            attention(1, 1, qkT, Vn)

            # wout chunks 2/3 ride in the dead qkT slot and a small
            # dedicated slot
            wos[2] = qp.tile([128, KSUB, 512], bf16, tag="qkT", name="wo2")
            nc.scalar.dma_start(wos[2][:], wout.ap()[:, :, 1024:1536])
            wos[3] = wo3p.tile([128, KSUB, 512], bf16, tag="wo3", name="wo3")
            nc.scalar.dma_start(wos[3][:], wout.ap()[:, :, 1536:2048])
            drain_ship()
            lhs1 = load_lhs(1, qp, "Vn")

            # all batch-0 groups run while the final A2As land; batch-1
            # m=0 groups need only the half-A payload (shipped mid-
            # attention), m=1 groups run last.
            for n in range(4):
                for m in range(SCW // 128):
                    outproj_group(0, n, m, lhs0, wos[n], 0)
            for m in range(SCW // 128):
                for n in range(4):
                    outproj_group(1, n, m, lhs1, wos[n], 0)

    nc.finalize()
    return nc


def _host_inputs(x, w_qkv, w_out):
    import ml_dtypes
    bf16 = ml_dtypes.bfloat16
    xTr = np.ascontiguousarray(
        x.reshape(B * S, D).T.reshape(KSUB, 128, B * S).transpose(1, 0, 2)
    ).astype(bf16)
    horder = [2 * i + hh for hh in range(HPC) for i in range(NCORES)]
    woutr = np.ascontiguousarray(
        w_out.reshape(H, HD, D)[horder].transpose(1, 0, 2)).astype(bf16)

    half = HD // 2
    inv = (1.0 / (ROPE_BASE ** (np.arange(half, dtype=np.float32) / half))
           ).astype(np.float32)
    ang = (np.arange(S, dtype=np.float32)[:, None] * inv[None, :])  # [S, 64]
    c = np.cos(ang).astype(np.float32).T      # [64, S]
    s = np.sin(ang).astype(np.float32).T
    cosg = np.ascontiguousarray(np.concatenate([c, c], axis=0)).astype(bf16)
    # pre-swapped: rows 0:64 = +sin (consumed against t[0:64] -> rt[64:128]),
    # rows 64:128 = -sin (consumed against t[64:128] -> rt[0:64])
    sing = np.ascontiguousarray(np.concatenate([s, -s], axis=0)).astype(bf16)

    # mneg[p, j] = 0 where j >= p else -1e9 (upper-tri of the diagonal
    # 128-block, padded to 512 query columns).
    u = np.arange(512)[None, :]
    p = np.arange(128)[:, None]
    mneg = np.where(u >= p, 0.0, -1e9).astype(bf16)

    maps = []
    for i in range(NCORES):
        h0, h1 = 2 * i, 2 * i + 1
        blocks = []
        for base in (0, D, 2 * D):
            blocks.append(w_qkv[:, base + 128 * h0:base + 128 * (h0 + 1)])
            blocks.append(w_qkv[:, base + 128 * h1:base + 128 * (h1 + 1)])
        shard = np.concatenate(blocks, axis=1)  # [D, 768]
        shard = np.ascontiguousarray(
            shard.reshape(KSUB, 128, 3 * HPC * HD).transpose(1, 0, 2)
        ).astype(bf16)
        maps.append({"xT": xTr, "wqkv": shard, "wout": woutr,
                     "cosg": cosg, "sing": sing, "mneg": mneg})
    return maps


def kernel(x, w_qkv, w_out):
    from concourse.bass_utils import run_bass_kernel_spmd

    x = np.asarray(x, dtype=np.float32)
    w_qkv = np.asarray(w_qkv, dtype=np.float32)
    w_out = np.asarray(w_out, dtype=np.float32)

    if "nc" not in _CACHE:
        _CACHE["nc"] = _build()
    nc = _CACHE["nc"]

    trace = bool(int(os.environ.get("KERNEL_TRACE", "0")))
    if trace:
        trace = _install_trace_shim()

    in_maps = _host_inputs(x, w_qkv, w_out)
    res = run_bass_kernel_spmd(nc, in_maps, core_ids=list(range(NCORES)),
                               trace=trace)
    _CACHE["last_result"] = res
    # y per core i: [B, 256, D]; rows 0:128 = query block i, rows
    # 128:256 = query block i+8 (per batch)
    full = np.empty((B * S, D), dtype=np.float32)
    for i in range(NCORES):
        yi = res.results[i]["y"]
        for b in range(B):
            full[b * S + i * 128: b * S + (i + 1) * 128] = yi[b, :128]
            full[b * S + 1024 + i * 128:
                 b * S + 1024 + (i + 1) * 128] = yi[b, 128:]
    return full.reshape(B, S, D)
